# revision 7
# baseline (speedup 1.0000x reference)
"""GCN (4x GCNConv + global_add_pool + MLP) on 8 Trainium2 NeuronCores.

Dataflow (per layer l): every node row of table_l = (D.h_{l-1}) @ W_l is
built on the owning core, AllGathered (fp16) into a replicated table, and
each core gathers the rows for the edges whose dst it owns (edges bucketed
by dst tile, 128 edge slots per PE matmul).  The scatter-add is a one-hot
matmul into PSUM.  All degree normalisation is folded into host-side data:
  - x is prescaled by D = diag(dinv) on the host and replicated chunk-wise
    (layer-1 table needs no comms; layer 1 aggregates 32-wide x, then
    applies W1).
  - one-hot values carry dinv[dst]^2 for layers 1..3 (one dinv for the GCN
    norm, one prescaling the layer output for the next table) and
    dinv[dst] for layer 4.
  - biases enter as a rank-1 PSUM-init matmul (dinv x b, or ones x b).
h is kept feature-major ([96, nodes]) so the next layer's table matmul
needs no transposes; layer 4 produces node-major h directly for pooling.
Each layer's AllGather is split in two halves (by node range) so half A
fires mid-way through the previous edge phase and half B overlaps the
half-A edge processing.  One-hot builds run on DVE (tensor_scalar
is_eq*mult, ~234ns) and the Activation engine (|iota-d| then
relu(dval-dval*t), ~300ns each) -- never on Pool/GpSimd, which executes
tensor_scalar ~10x slower.
"""

import math
import os

import numpy as np

P = 128
NFEAT = 32
HID = 96
NG = 2048          # graphs
NCORES = 8
NPC = 12544        # nodes per core (98 * 128)
NT = NPC // P      # 98 node tiles per core
NPAD = NPC * NCORES
HALF_T = NT // 2   # 49 tiles in chunk A
HALF_N = HALF_T * P
NAB = NCORES * HALF_N
NZROW = NG + 512   # pool scatter buffer rows
ACT_EVERY = 4      # every 4th one-hot built on the Act engine

F16 = np.float16


# ----------------------------------------------------------------------------
# Host-side preprocessing.
# ----------------------------------------------------------------------------

def _edge_grids(s, d, dv2, dv1, M):
    """Pack per-core edges (sorted by local dst) into [P, NT*M] slot grids."""
    t_e = d // P
    cnt = np.bincount(t_e, minlength=NT)
    EPT = M * P
    srcg = np.zeros((NT, EPT), np.int32)
    doff = np.full((NT, EPT), -1.0, np.float32)
    dval2 = np.zeros((NT, EPT), np.float32)
    dval1 = np.zeros((NT, EPT), np.float32)
    start = np.zeros(NT + 1, np.int64)
    start[1:] = np.cumsum(cnt)
    slot = np.arange(len(d)) - start[t_e]
    flat = t_e * EPT + slot
    srcg.reshape(-1)[flat] = s.astype(np.int32)
    doff.reshape(-1)[flat] = (d - t_e * P).astype(np.float32)
    dval2.reshape(-1)[flat] = dv2
    dval1.reshape(-1)[flat] = dv1

    def dev(a):  # [NT, M*P] -> [P, NT*M]  (slot col = t*M + m)
        return np.ascontiguousarray(
            a.reshape(NT, M, P).transpose(2, 0, 1).reshape(P, NT * M)
        )

    return dev(srcg), dev(doff), dev(dval2), dev(dval1)


def _prep(x, edge_index, batch):
    N = x.shape[0]
    src = np.asarray(edge_index[0], dtype=np.int64)
    dst = np.asarray(edge_index[1], dtype=np.int64)
    loops = np.arange(N, dtype=np.int64)
    src = np.concatenate([src, loops])
    dst = np.concatenate([dst, loops])
    deg = np.bincount(dst, minlength=N).astype(np.float64)
    dinv = np.where(deg > 0, 1.0 / np.sqrt(np.maximum(deg, 1.0)), 0.0).astype(
        np.float32
    )
    batch = np.asarray(batch, dtype=np.int64)

    xhat = np.zeros((NPAD, NFEAT), F16)
    xhat[:N] = (np.asarray(x, np.float32) * dinv[:, None]).astype(F16)
    # chunk-ordered copies of the replicated layer-1 table
    xr = xhat.reshape(NCORES, NPC, NFEAT)
    xhatA = np.ascontiguousarray(xr[:, :HALF_N].reshape(NAB, NFEAT))
    xhatB = np.ascontiguousarray(xr[:, HALF_N:].reshape(NAB, NFEAT))

    dv2_all = (dinv.astype(np.float64) ** 2).astype(np.float32)
    dv1_all = dinv.astype(np.float32)

    owner = dst // NPC
    percore = []
    MA = MB = 1
    GT = 1
    for c in range(NCORES):
        m = owner == c
        s_c = src[m]
        d_c = dst[m] - c * NPC
        o = np.argsort(d_c, kind="stable")
        s_c, d_c = s_c[o], d_c[o]
        dv2 = dv2_all[dst[m]][o]
        dv1 = dv1_all[dst[m]][o]
        s_own = s_c // NPC
        s_loc = s_c - s_own * NPC
        in_a = s_loc < HALF_N
        rows_a = s_own * HALF_N + s_loc
        rows_b = s_own * HALF_N + (s_loc - HALF_N)
        percore.append(
            dict(
                a=(rows_a[in_a], d_c[in_a], dv2[in_a], dv1[in_a]),
                b=(rows_b[~in_a], d_c[~in_a], dv2[~in_a], dv1[~in_a]),
            )
        )
        for key in ("a", "b"):
            d_k = percore[-1][key][1]
            mx = int(np.bincount(d_k // P, minlength=NT).max())
            need = max(1, math.ceil(mx / P))
            if key == "a":
                MA = max(MA, need)
            else:
                MB = max(MB, need)
        n0 = c * NPC
        nreal = min(NPC, max(0, N - n0))
        if nreal > 0:
            gb = int(batch[n0])
            gmax = int(batch[n0 + nreal - 1])
            GT = max(GT, math.ceil((gmax - gb + 1) / P))

    inputs = []
    for c in range(NCORES):
        pc = percore[c]
        sgA, doA, dv2A, dv1A = _edge_grids(*pc["a"], MA)
        sgB, doB, dv2B, dv1B = _edge_grids(*pc["b"], MB)

        n0 = c * NPC
        nreal = min(NPC, max(0, N - n0))
        dloc = np.zeros(NPC, F16)
        dloc[:nreal] = dinv[n0 : n0 + nreal].astype(F16)
        gbase = int(batch[n0]) if nreal > 0 else 0
        pg = np.full(NPC, 30000.0, np.float32)
        pg[:nreal] = (batch[n0 : n0 + nreal] - gbase).astype(np.float32)
        growidx = (
            gbase
            + np.arange(GT, dtype=np.int32)[None, :] * P
            + np.arange(P, dtype=np.int32)[:, None]
        ).astype(np.int32)

        inputs.append(
            dict(
                xhata=xhatA, xhatb=xhatB,
                srcga=sgA, doffa=doA, dval2a=dv2A, dval1a=dv1A,
                srcgb=sgB, doffb=doB, dval2b=dv2B, dval1b=dv1B,
                dinvrow=np.ascontiguousarray(dloc.reshape(1, NPC)),
                poolg=np.ascontiguousarray(pg.reshape(NT, P).T),
                growidx=growidx,
            )
        )
    return inputs, MA, MB, GT


def _weights16(W1, b1, W2, b2, W3, b3, W4, b4, Wf1, bf1, Wf2):
    return dict(
        w1=np.asarray(W1, np.float32).astype(F16),
        w2=np.asarray(W2, np.float32).astype(F16),
        w3=np.asarray(W3, np.float32).astype(F16),
        w4=np.asarray(W4, np.float32).astype(F16),
        b1=np.asarray(b1, np.float32).astype(F16).reshape(1, HID),
        b2=np.asarray(b2, np.float32).astype(F16).reshape(1, HID),
        b3=np.asarray(b3, np.float32).astype(F16).reshape(1, HID),
        b4=np.asarray(b4, np.float32).astype(F16).reshape(1, HID),
        wf1=np.asarray(Wf1, np.float32).astype(F16),
        bf1col=np.asarray(bf1, np.float32).reshape(NFEAT, 1),
        wf2=np.asarray(Wf2, np.float32).astype(F16).reshape(NFEAT, 1),
    )


# ----------------------------------------------------------------------------
# Numpy emulation of the device program (index plumbing + fp16 effects).
# ----------------------------------------------------------------------------

def _emulate(inputs, MA, MB, GT, w, bf2val):
    f32 = np.float32
    iota = np.arange(P, dtype=F16)

    def onehot(doff, dval, k):
        return (
            (iota[None, :] == doff[:, k : k + 1]).astype(F16)
            * dval[:, k : k + 1]
        ).astype(F16)

    def agg_pass(table, inp, sg, do, dv, M, width):
        # per-tile [width, P] partial sums (f32), transposed orientation
        out = np.zeros((NT, width, P), f32)
        for t in range(NT):
            for m in range(M):
                k = t * M + m
                msg = table[sg[:, k]]
                oh = onehot(do, dv, k)
                out[t] += msg.astype(f32).T @ oh.astype(f32)
        return out

    hT = [None] * NCORES
    xhatA = inputs[0]["xhata"]
    xhatB = inputs[0]["xhatb"]
    for c in range(NCORES):
        inp = inputs[c]
        dinvrow = inp["dinvrow"][0].astype(f32)
        aggA = agg_pass(xhatA, inp, inp["srcga"], inp["doffa"], inp["dval2a"],
                        MA, NFEAT)
        aggB = agg_pass(xhatB, inp, inp["srcgb"], inp["doffb"], inp["dval2b"],
                        MB, NFEAT)
        hTc = np.zeros((HID, NPC), F16)
        for t in range(NT):
            aggT = (aggA[t] + aggB[t]).astype(F16)
            ps = w["b1"].astype(f32).T @ dinvrow[None, t * P : (t + 1) * P]
            ps = ps + w["w1"].astype(f32).T @ aggT.astype(f32)
            hTc[:, t * P : (t + 1) * P] = np.maximum(ps, 0).astype(F16)
        hT[c] = hTc

    h4 = [None] * NCORES
    for l, wl, bl in ((2, "w2", "b2"), (3, "w3", "b3"), (4, "w4", "b4")):
        tabA = np.zeros((NAB, HID), F16)
        tabB = np.zeros((NAB, HID), F16)
        for c in range(NCORES):
            contrib = (hT[c].astype(f32).T @ w[wl].astype(f32)).astype(F16)
            tabA[c * HALF_N : (c + 1) * HALF_N] = contrib[:HALF_N]
            tabB[c * HALF_N : (c + 1) * HALF_N] = contrib[HALF_N:]
        for c in range(NCORES):
            inp = inputs[c]
            dinvrow = inp["dinvrow"][0].astype(f32)
            dva, dvb = (("dval2a", "dval2b") if l < 4 else ("dval1a", "dval1b"))
            newT = np.zeros((HID, NPC), F16)
            newU = np.zeros((NPC, HID), F16)
            accA = agg_pass(tabA, inp, inp["srcga"], inp["doffa"], inp[dva],
                            MA, HID).astype(F16)  # parked fp16
            accB = agg_pass(tabB, inp, inp["srcgb"], inp["doffb"], inp[dvb],
                            MB, HID)
            for t in range(NT):
                if l < 4:
                    ps = w[bl].astype(f32).T @ dinvrow[None, t * P : (t + 1) * P]
                else:
                    ps = np.repeat(w[bl].astype(f32).T, P, axis=1)
                tot = ps + accA[t].astype(f32) + accB[t]
                if l < 4:
                    newT[:, t * P : (t + 1) * P] = np.maximum(tot, 0).astype(F16)
                else:
                    newU[t * P : (t + 1) * P] = np.maximum(tot, 0).astype(F16).T
            if l < 4:
                hT[c] = newT
            else:
                h4[c] = newU

    zbuf = np.zeros((NZROW, NFEAT), f32)
    for c in range(NCORES):
        inp = inputs[c]
        for g in range(GT):
            gT = np.zeros((HID, P), f32)
            giota = (np.arange(P, dtype=np.float32) + g * P).astype(F16)
            for t in range(NT):
                ohp = (giota[None, :] == inp["poolg"][:, t : t + 1]).astype(F16)
                gT += h4[c][t * P : (t + 1) * P].astype(f32).T @ ohp.astype(f32)
            gT16 = gT.astype(F16)
            zpreT = w["wf1"].astype(f32).T @ gT16.astype(f32)  # [32, P]
            zbuf[inp["growidx"][:, g]] += zpreT.T
    z = np.maximum(zbuf[:NG] + w["bf1col"].T, 0).astype(F16)
    out = z.astype(f32) @ w["wf2"].astype(f32) + bf2val
    return out.reshape(NG, 1)


# ----------------------------------------------------------------------------
# Bass program.
# ----------------------------------------------------------------------------

def _build_program(MA, MB, GT, bf2val):
    from concourse import bacc, bass, mybir, tile
    from concourse.masks import make_identity

    f32 = mybir.dt.float32
    f16 = mybir.dt.float16
    i32 = mybir.dt.int32
    AF = mybir.ActivationFunctionType
    OP = mybir.AluOpType

    nc = bacc.Bacc("TRN2", target_bir_lowering=False, debug=False)

    xhatA_p = nc.declare_dram_parameter("xhata", [NAB, NFEAT], f16, isOutput=False)
    xhatB_p = nc.declare_dram_parameter("xhatb", [NAB, NFEAT], f16, isOutput=False)
    sgA_p = nc.declare_dram_parameter("srcga", [P, NT * MA], i32, isOutput=False)
    doA_p = nc.declare_dram_parameter("doffa", [P, NT * MA], f32, isOutput=False)
    dv2A_p = nc.declare_dram_parameter("dval2a", [P, NT * MA], f32, isOutput=False)
    dv1A_p = nc.declare_dram_parameter("dval1a", [P, NT * MA], f32, isOutput=False)
    sgB_p = nc.declare_dram_parameter("srcgb", [P, NT * MB], i32, isOutput=False)
    doB_p = nc.declare_dram_parameter("doffb", [P, NT * MB], f32, isOutput=False)
    dv2B_p = nc.declare_dram_parameter("dval2b", [P, NT * MB], f32, isOutput=False)
    dv1B_p = nc.declare_dram_parameter("dval1b", [P, NT * MB], f32, isOutput=False)
    dinv_p = nc.declare_dram_parameter("dinvrow", [1, NPC], f16, isOutput=False)
    poolg_p = nc.declare_dram_parameter("poolg", [P, NT], f32, isOutput=False)
    grow_p = nc.declare_dram_parameter("growidx", [P, GT], i32, isOutput=False)
    w_ps = {
        "w1": nc.declare_dram_parameter("w1", [NFEAT, HID], f16, isOutput=False),
        "w2": nc.declare_dram_parameter("w2", [HID, HID], f16, isOutput=False),
        "w3": nc.declare_dram_parameter("w3", [HID, HID], f16, isOutput=False),
        "w4": nc.declare_dram_parameter("w4", [HID, HID], f16, isOutput=False),
        "b1": nc.declare_dram_parameter("b1", [1, HID], f16, isOutput=False),
        "b2": nc.declare_dram_parameter("b2", [1, HID], f16, isOutput=False),
        "b3": nc.declare_dram_parameter("b3", [1, HID], f16, isOutput=False),
        "b4": nc.declare_dram_parameter("b4", [1, HID], f16, isOutput=False),
        "wf1": nc.declare_dram_parameter("wf1", [HID, NFEAT], f16, isOutput=False),
        "bf1col": nc.declare_dram_parameter("bf1col", [NFEAT, 1], f32, isOutput=False),
        "wf2": nc.declare_dram_parameter("wf2", [NFEAT, 1], f16, isOutput=False),
    }
    out_p = nc.declare_dram_parameter("out", [1, NG], f32, isOutput=True)

    groups = [list(range(NCORES))]
    MM = max(MA, MB)

    with tile.TileContext(nc) as tc:
        with (
            tc.tile_pool(name="const", bufs=1) as cp,
            tc.tile_pool(name="sb", bufs=1) as sb,
            tc.tile_pool(name="ps", bufs=2, space="PSUM") as ps,
            tc.tile_pool(name="dram", bufs=1, space="DRAM") as dp,
        ):
            # ---- persistent SBUF ----------------------------------------
            hT = cp.tile([HID, NT, P], f16)
            accT = cp.tile([HID, NT, P], f16)   # chunk-A parking, layers 2-3
            h4 = cp.tile([P, NT, HID], f16)
            acc4 = cp.tile([P, NT, HID], f16)   # chunk-A parking, layer 4
            sgA_sb = cp.tile([P, NT * MA], i32)
            doA_sb = cp.tile([P, NT * MA], f32)
            dv2A_sb = cp.tile([P, NT * MA], f32)
            dv1A_sb = cp.tile([P, NT * MA], f32)
            ndv2A_sb = cp.tile([P, NT * MA], f32)
            ndv1A_sb = cp.tile([P, NT * MA], f32)
            sgB_sb = cp.tile([P, NT * MB], i32)
            doB_sb = cp.tile([P, NT * MB], f32)
            dv2B_sb = cp.tile([P, NT * MB], f32)
            dv1B_sb = cp.tile([P, NT * MB], f32)
            ndv2B_sb = cp.tile([P, NT * MB], f32)
            ndv1B_sb = cp.tile([P, NT * MB], f32)
            dinv_sb = cp.tile([1, NPC], f16)
            poolg_sb = cp.tile([P, NT], f32)
            grow_sb = cp.tile([P, GT], i32)
            iota_i = cp.tile([P, GT * P], i32)
            iota_f = cp.tile([P, GT * P], f16)
            ident = cp.tile([P, P], f32)
            ones_row = cp.tile([1, P], f16)
            zero32 = cp.tile([P, NFEAT], f32)
            w_sb = {
                k: cp.tile(list(pshape), f16, name=k + "sb")
                for k, pshape in (
                    ("w1", (NFEAT, HID)), ("w2", (HID, HID)),
                    ("w3", (HID, HID)), ("w4", (HID, HID)),
                    ("b1", (1, HID)), ("b2", (1, HID)),
                    ("b3", (1, HID)), ("b4", (1, HID)),
                    ("wf1", (HID, NFEAT)), ("wf2", (NFEAT, 1)),
                )
            }
            bf1_sb = cp.tile([NFEAT, 1], f32)
            osb = cp.tile([1, NG], f32)

            # ---- DRAM scratch -------------------------------------------
            ctbA = [dp.tile([HALF_N, HID], f16, name=f"ctbA{l}") for l in range(3)]
            ctbB = [dp.tile([HALF_N, HID], f16, name=f"ctbB{l}") for l in range(3)]
            tabA = [
                dp.tile([NAB, HID], f16, name=f"tabA{l}", addr_space="Shared")
                for l in range(3)
            ]
            tabB = [
                dp.tile([NAB, HID], f16, name=f"tabB{l}", addr_space="Shared")
                for l in range(3)
            ]
            zbuf = dp.tile([NZROW, NFEAT], f32)
            zred = dp.tile([NZROW, NFEAT], f32, addr_space="Shared")

            # ---- load constants -----------------------------------------
            for dst_sb, src_p in (
                (sgA_sb, sgA_p), (doA_sb, doA_p), (dv2A_sb, dv2A_p),
                (dv1A_sb, dv1A_p),
                (sgB_sb, sgB_p), (doB_sb, doB_p), (dv2B_sb, dv2B_p),
                (dv1B_sb, dv1B_p),
                (dinv_sb, dinv_p), (poolg_sb, poolg_p), (grow_sb, grow_p),
                (bf1_sb, w_ps["bf1col"]),
            ):
                nc.sync.dma_start(out=dst_sb[:], in_=src_p[:])
            for k in w_sb:
                nc.sync.dma_start(out=w_sb[k][:], in_=w_ps[k][:])

            make_identity(nc, ident[:])
            nc.gpsimd.iota(
                iota_i[:], pattern=[[1, GT * P]], base=0, channel_multiplier=0
            )
            nc.vector.tensor_copy(out=iota_f[:], in_=iota_i[:])
            nc.vector.memset(ones_row[:], 1.0)
            nc.vector.memset(zero32[:], 0.0)
            # negated one-hot values for the Act-engine relu trick
            for dst_sb, src_sb in (
                (ndv2A_sb, dv2A_sb), (ndv1A_sb, dv1A_sb),
                (ndv2B_sb, dv2B_sb), (ndv1B_sb, dv1B_sb),
            ):
                nc.vector.tensor_scalar(
                    out=dst_sb[:], in0=src_sb[:], scalar1=-1.0, scalar2=None,
                    op0=OP.mult,
                )

            # zero the pool scatter buffer early (sync queue)
            for r in range(NZROW // P):
                nc.sync.dma_start(out=zbuf[r * P : (r + 1) * P, :], in_=zero32[:])

            def onehots(oh, do_sb, dv_sb, ndv_sb, t, M):
                # DVE: fused is_eq*dval; Act: |iota-d| then relu(dval-dval*t)
                for m in range(M):
                    if m % ACT_EVERY == ACT_EVERY - 1:
                        tmp = sb.tile([P, P], f16, tag="ohtmp", bufs=3)
                        nc.scalar.activation(
                            out=tmp[:], in_=iota_f[:, 0:P], func=AF.Abs,
                            bias=do_sb[:, t * M + m : t * M + m + 1],
                            scale=-1.0,
                        )
                        nc.scalar.activation(
                            out=oh[:, m * P : (m + 1) * P], in_=tmp[:],
                            func=AF.Relu,
                            bias=dv_sb[:, t * M + m : t * M + m + 1],
                            scale=ndv_sb[:, t * M + m : t * M + m + 1],
                        )
                    else:
                        nc.vector.tensor_scalar(
                            out=oh[:, m * P : (m + 1) * P],
                            in0=iota_f[:, 0:P],
                            scalar1=do_sb[:, t * M + m : t * M + m + 1],
                            scalar2=dv_sb[:, t * M + m : t * M + m + 1],
                            op0=OP.is_equal,
                            op1=OP.mult,
                        )

            def emit_table(li, t):
                # contrib tile for table index li (= layer li+2) from hT[:, t, :]
                wl = ("w2", "w3", "w4")[li]
                pc = ps.tile([P, HID], f32, tag="tbl")
                nc.tensor.matmul(
                    out=pc[:], lhsT=hT[:, t, :], rhs=w_sb[wl][:],
                    start=True, stop=True,
                )
                cs = sb.tile([P, HID], f16, tag="cs", bufs=3)
                nc.vector.tensor_copy(out=cs[:], in_=pc[:])
                if t < HALF_T:
                    nc.sync.dma_start(
                        out=ctbA[li][t * P : (t + 1) * P, :], in_=cs[:]
                    )
                else:
                    nc.sync.dma_start(
                        out=ctbB[li][(t - HALF_T) * P : (t - HALF_T + 1) * P, :],
                        in_=cs[:],
                    )
                if t == HALF_T - 1:
                    nc.gpsimd.collective_compute(
                        "AllGather", OP.bypass, replica_groups=groups,
                        ins=[ctbA[li][:]], outs=[tabA[li][:]],
                    )
                if t == NT - 1:
                    nc.gpsimd.collective_compute(
                        "AllGather", OP.bypass, replica_groups=groups,
                        ins=[ctbB[li][:]], outs=[tabB[li][:]],
                    )

            # ================= layer 1 (local chunked x-hat table) =======
            for t in range(NT):
                msgA = sb.tile([P, MM * NFEAT], f16, tag="msg1a", bufs=3)
                nc.gpsimd.indirect_dma_start(
                    out=msgA[:, 0 : MA * NFEAT],
                    out_offset=None,
                    in_=xhatA_p[:],
                    in_offset=bass.IndirectOffsetOnAxis(
                        ap=sgA_sb[:, t * MA : (t + 1) * MA], axis=0
                    ),
                )
                msgB = sb.tile([P, MM * NFEAT], f16, tag="msg1b", bufs=3)
                nc.gpsimd.indirect_dma_start(
                    out=msgB[:, 0 : MB * NFEAT],
                    out_offset=None,
                    in_=xhatB_p[:],
                    in_offset=bass.IndirectOffsetOnAxis(
                        ap=sgB_sb[:, t * MB : (t + 1) * MB], axis=0
                    ),
                )
                ohA = sb.tile([P, MM * P], f16, tag="oha", bufs=3)
                onehots(ohA, doA_sb, dv2A_sb, ndv2A_sb, t, MA)
                ohB = sb.tile([P, MM * P], f16, tag="ohb", bufs=3)
                onehots(ohB, doB_sb, dv2B_sb, ndv2B_sb, t, MB)
                agg = ps.tile([NFEAT, P], f32, tag="agg")
                nmm = MA + MB
                i = 0
                for msg, oh, M, W in ((msgA, ohA, MA, NFEAT), (msgB, ohB, MB, NFEAT)):
                    for m in range(M):
                        nc.tensor.matmul(
                            out=agg[:],
                            lhsT=msg[:, m * W : (m + 1) * W],
                            rhs=oh[:, m * P : (m + 1) * P],
                            start=(i == 0),
                            stop=(i == nmm - 1),
                        )
                        i += 1
                aggT = sb.tile([NFEAT, P], f16, tag="aggT", bufs=3)
                nc.vector.tensor_copy(out=aggT[:], in_=agg[:])
                ph = ps.tile([HID, P], f32, tag="hT")
                nc.tensor.matmul(
                    out=ph[:],
                    lhsT=w_sb["b1"][:],
                    rhs=dinv_sb[0:1, t * P : (t + 1) * P],
                    start=True,
                    stop=False,
                )
                nc.tensor.matmul(
                    out=ph[:], lhsT=w_sb["w1"][:], rhs=aggT[:],
                    start=False, stop=True,
                )
                nc.scalar.activation(out=hT[:, t, :], in_=ph[:], func=AF.Relu)
                emit_table(0, t)

            # ================= layers 2..4 ===============================
            for li, blname in enumerate(("b2", "b3", "b4")):
                last = li == 2
                dva_sb = dv2A_sb if not last else dv1A_sb
                dvb_sb = dv2B_sb if not last else dv1B_sb
                ndva_sb = ndv2A_sb if not last else ndv1A_sb
                ndvb_sb = ndv2B_sb if not last else ndv1B_sb
                # pass A: park chunk-A partial sums
                for t in range(NT):
                    msg = sb.tile([P, MM * HID], f16, tag="msg", bufs=3)
                    nc.gpsimd.indirect_dma_start(
                        out=msg[:, 0 : MA * HID],
                        out_offset=None,
                        in_=tabA[li][:],
                        in_offset=bass.IndirectOffsetOnAxis(
                            ap=sgA_sb[:, t * MA : (t + 1) * MA], axis=0
                        ),
                    )
                    oh = sb.tile([P, MM * P], f16, tag="oha", bufs=3)
                    onehots(oh, doA_sb, dva_sb, ndva_sb, t, MA)
                    if not last:
                        pa = ps.tile([HID, P], f32, tag="hT")
                        for m in range(MA):
                            nc.tensor.matmul(
                                out=pa[:],
                                lhsT=msg[:, m * HID : (m + 1) * HID],
                                rhs=oh[:, m * P : (m + 1) * P],
                                start=(m == 0),
                                stop=(m == MA - 1),
                            )
                        nc.vector.tensor_copy(out=accT[:, t, :], in_=pa[:])
                    else:
                        pa = ps.tile([P, HID], f32, tag="tbl")
                        for m in range(MA):
                            nc.tensor.matmul(
                                out=pa[:],
                                lhsT=oh[:, m * P : (m + 1) * P],
                                rhs=msg[:, m * HID : (m + 1) * HID],
                                start=(m == 0),
                                stop=(m == MA - 1),
                            )
                        nc.vector.tensor_copy(out=acc4[:, t, :], in_=pa[:])
                # pass B: finish, add bias via PSUM-init matmul, relu;
                # interleave the next layer's contrib/AllGather.
                for t in range(NT):
                    msg = sb.tile([P, MM * HID], f16, tag="msg", bufs=3)
                    nc.gpsimd.indirect_dma_start(
                        out=msg[:, 0 : MB * HID],
                        out_offset=None,
                        in_=tabB[li][:],
                        in_offset=bass.IndirectOffsetOnAxis(
                            ap=sgB_sb[:, t * MB : (t + 1) * MB], axis=0
                        ),
                    )
                    oh = sb.tile([P, MM * P], f16, tag="ohb", bufs=3)
                    onehots(oh, doB_sb, dvb_sb, ndvb_sb, t, MB)
                    if not last:
                        pb = ps.tile([HID, P], f32, tag="hT")
                        nc.tensor.matmul(
                            out=pb[:],
                            lhsT=w_sb[blname][:],
                            rhs=dinv_sb[0:1, t * P : (t + 1) * P],
                            start=True,
                            stop=False,
                        )
                        for m in range(MB):
                            nc.tensor.matmul(
                                out=pb[:],
                                lhsT=msg[:, m * HID : (m + 1) * HID],
                                rhs=oh[:, m * P : (m + 1) * P],
                                start=False,
                                stop=(m == MB - 1),
                            )
                        nc.vector.tensor_tensor(
                            out=pb[:], in0=pb[:], in1=accT[:, t, :], op=OP.add
                        )
                        nc.scalar.activation(
                            out=hT[:, t, :], in_=pb[:], func=AF.Relu
                        )
                        emit_table(li + 1, t)
                    else:
                        pb = ps.tile([P, HID], f32, tag="tbl")
                        nc.tensor.matmul(
                            out=pb[:],
                            lhsT=ones_row[:],
                            rhs=w_sb[blname][:],
                            start=True,
                            stop=False,
                        )
                        for m in range(MB):
                            nc.tensor.matmul(
                                out=pb[:],
                                lhsT=oh[:, m * P : (m + 1) * P],
                                rhs=msg[:, m * HID : (m + 1) * HID],
                                start=False,
                                stop=(m == MB - 1),
                            )
                        nc.vector.tensor_tensor(
                            out=pb[:], in0=pb[:], in1=acc4[:, t, :], op=OP.add
                        )
                        nc.scalar.activation(
                            out=h4[:, t, :], in_=pb[:], func=AF.Relu
                        )

            # ================= pool + head ===============================
            for g in range(GT):
                pg_ps = ps.tile([HID, P], f32, tag="hT")
                for t in range(NT):
                    ohp = sb.tile([P, P], f16, tag="ohp", bufs=4)
                    nc.vector.tensor_scalar(
                        out=ohp[:],
                        in0=iota_f[:, g * P : (g + 1) * P],
                        scalar1=poolg_sb[:, t : t + 1],
                        scalar2=None,
                        op0=OP.is_equal,
                    )
                    nc.tensor.matmul(
                        out=pg_ps[:],
                        lhsT=h4[:, t, :],
                        rhs=ohp[:],
                        start=(t == 0),
                        stop=(t == NT - 1),
                    )
                gT = sb.tile([HID, P], f16, tag="gT", bufs=2)
                nc.vector.tensor_copy(out=gT[:], in_=pg_ps[:])
                pz = ps.tile([NFEAT, P], f32, tag="agg")
                nc.tensor.matmul(
                    out=pz[:], lhsT=w_sb["wf1"][:], rhs=gT[:],
                    start=True, stop=True,
                )
                zT = sb.tile([NFEAT, P], f32, tag="zT", bufs=2)
                nc.vector.tensor_copy(out=zT[:], in_=pz[:])
                pzt = ps.tile([P, NFEAT], f32, tag="tbl")
                nc.tensor.transpose(
                    out=pzt[:], in_=zT[:], identity=ident[0:NFEAT, 0:NFEAT]
                )
                zsb = sb.tile([P, NFEAT], f32, tag="zsb", bufs=2)
                nc.vector.tensor_copy(out=zsb[:], in_=pzt[:])
                nc.gpsimd.indirect_dma_start(
                    out=zbuf[:],
                    out_offset=bass.IndirectOffsetOnAxis(
                        ap=grow_sb[:, g : g + 1], axis=0
                    ),
                    in_=zsb[:],
                    in_offset=None,
                )

            nc.gpsimd.collective_compute(
                "AllReduce", OP.add, replica_groups=groups,
                ins=[zbuf[:]], outs=[zred[:]],
            )

            for j in range(NG // P):
                zr = sb.tile([P, NFEAT], f32, tag="zsb", bufs=2)
                nc.sync.dma_start(out=zr[:], in_=zred[j * P : (j + 1) * P, :])
                pt = ps.tile([NFEAT, P], f32, tag="agg")
                nc.tensor.transpose(out=pt[:], in_=zr[:], identity=ident[:])
                zrel = sb.tile([NFEAT, P], f16, tag="zrel", bufs=2)
                nc.scalar.activation(
                    out=zrel[:], in_=pt[:], func=AF.Relu, bias=bf1_sb[:, 0:1]
                )
                po = ps.tile([1, P], f32, tag="out")
                nc.tensor.matmul(
                    out=po[:], lhsT=w_sb["wf2"][:], rhs=zrel[:],
                    start=True, stop=True,
                )
                nc.vector.tensor_scalar(
                    out=osb[:, j * P : (j + 1) * P],
                    in0=po[:],
                    scalar1=float(bf2val),
                    scalar2=None,
                    op0=OP.add,
                )
            nc.sync.dma_start(out=out_p[:], in_=osb[:])

    nc.finalize()
    return nc


# ----------------------------------------------------------------------------
# Entry point.
# ----------------------------------------------------------------------------

_RUN_KWARGS = {}
LAST_RESULTS = None


def kernel(
    x, edge_index, batch,
    W1, b1, W2, b2, W3, b3, W4, b4, Wf1, bf1, Wf2, bf2,
):
    from concourse.bass_utils import run_bass_kernel_spmd

    inputs, MA, MB, GT = _prep(
        np.asarray(x), np.asarray(edge_index), np.asarray(batch)
    )
    bf2val = float(np.asarray(bf2).reshape(-1)[0])
    w16 = _weights16(W1, b1, W2, b2, W3, b3, W4, b4, Wf1, bf1, Wf2)

    if os.environ.get("BASS_EMULATE"):
        return _emulate(inputs, MA, MB, GT, w16, bf2val).astype(np.float32)

    nc = _build_program(MA, MB, GT, bf2val)
    in_maps = [{**inputs[c], **w16} for c in range(NCORES)]
    res = run_bass_kernel_spmd(
        nc, in_maps, core_ids=list(range(NCORES)), **_RUN_KWARGS
    )
    global LAST_RESULTS
    LAST_RESULTS = res
    out = np.asarray(res.results[0]["out"]).reshape(NG, 1).astype(np.float32)
    return out


# revision 12
# speedup vs baseline: 1.1458x; 1.1458x over previous
"""GCN (4x GCNConv + global_add_pool + MLP) on 8 Trainium2 NeuronCores.

Per layer l: node rows of table_l = (D.h_{l-1}) @ W_l are built on the
owning core (fp8), AllGathered into a replicated table, and each core
gathers the rows for the edges whose dst it owns (edges bucketed by dst
tile, 128 edge slots per PE matmul).  The scatter-add is a one-hot matmul
into PSUM; the 0/1 one-hot matrices are HOST-PRECOMPUTED in fp8 (exact)
and streamed from DRAM, so no engine builds them on device.  Degree
normalisation (dinv[dst]^2 for layers 1-3 output prescaling + GCN norm,
dinv[dst] for layer 4; dinv[src] lives inside the prescaled tables) is
applied after the PSUM accumulation: layers 1-3 multiply by a persistent
[96, nodes] dinv^2 broadcast (DVE), layer 4 (node-major PSUM) uses the
Activation engine's per-partition scale.  Biases enter as a rank-1
PSUM-init matmul (b x sqrt(deg), cancelling the post-scale).
h is kept feature-major so table matmuls need no transposes; layer 4
produces node-major h directly for pooling.  Each AllGather is split in
two halves so half A fires mid-way through the previous edge phase and
half B overlaps the half-A edge processing.
"""

import math
import os

import numpy as np
import ml_dtypes

P = 128
NFEAT = 32
HID = 96
NG = 2048          # graphs
NCORES = 8
NPC = 12544        # nodes per core (98 * 128)
NT = NPC // P      # 98 node tiles per core
NPAD = NPC * NCORES
HALF_T = NT // 2   # 49 tiles in chunk A
HALF_N = HALF_T * P
NAB = NCORES * HALF_N
NZROW = NG + 512   # pool scatter buffer rows

F16 = np.float16
F8 = ml_dtypes.float8_e4m3fn


# ----------------------------------------------------------------------------
# Host-side preprocessing.
# ----------------------------------------------------------------------------

def _edge_grids(s, d, M):
    """Pack per-core edges (sorted by local dst) into slot grids.

    Returns srcg [P, NT*M] i32 and the fp8 0/1 one-hot stream
    [P, NT*M*P] (slot col t*M+m, one-hot over the 128 dst offsets).
    """
    t_e = d // P
    cnt = np.bincount(t_e, minlength=NT)
    EPT = M * P
    srcg = np.zeros((NT, EPT), np.int32)
    doff = np.full((NT, EPT), -1, np.int32)
    start = np.zeros(NT + 1, np.int64)
    start[1:] = np.cumsum(cnt)
    slot = np.arange(len(d)) - start[t_e]
    flat = t_e * EPT + slot
    srcg.reshape(-1)[flat] = s.astype(np.int32)
    doff.reshape(-1)[flat] = (d - t_e * P).astype(np.int32)

    def dev(a):  # [NT, M*P] -> [P, NT*M]
        return np.ascontiguousarray(
            a.reshape(NT, M, P).transpose(2, 0, 1).reshape(P, NT * M)
        )

    srcg_d = dev(srcg)
    doff_d = dev(doff)  # [P, NT*M]
    oh = np.zeros((P, NT * M, P), F8)
    pp, kk = np.nonzero(doff_d >= 0)
    oh[pp, kk, doff_d[pp, kk]] = F8(1.0)
    return srcg_d, np.ascontiguousarray(oh.reshape(P, NT * M * P))


def _prep(x, edge_index, batch):
    N = x.shape[0]
    src = np.asarray(edge_index[0], dtype=np.int64)
    dst = np.asarray(edge_index[1], dtype=np.int64)
    loops = np.arange(N, dtype=np.int64)
    src = np.concatenate([src, loops])
    dst = np.concatenate([dst, loops])
    deg = np.bincount(dst, minlength=N).astype(np.float64)
    dinv = np.where(deg > 0, 1.0 / np.sqrt(np.maximum(deg, 1.0)), 0.0).astype(
        np.float32
    )
    sqdg = np.where(dinv > 0, 1.0 / np.maximum(dinv, 1e-30), 0.0).astype(
        np.float32
    )
    batch = np.asarray(batch, dtype=np.int64)

    xhat = np.zeros((NPAD, NFEAT), F8)
    xhat[:N] = (np.asarray(x, np.float32) * dinv[:, None]).astype(F8)
    xr = xhat.reshape(NCORES, NPC, NFEAT)
    xhatA = np.ascontiguousarray(xr[:, :HALF_N].reshape(NAB, NFEAT))
    xhatB = np.ascontiguousarray(xr[:, HALF_N:].reshape(NAB, NFEAT))

    owner = dst // NPC
    percore = []
    MA = MB = 1
    GT = 1
    for c in range(NCORES):
        m = owner == c
        s_c = src[m]
        d_c = dst[m] - c * NPC
        o = np.argsort(d_c, kind="stable")
        s_c, d_c = s_c[o], d_c[o]
        s_own = s_c // NPC
        s_loc = s_c - s_own * NPC
        in_a = s_loc < HALF_N
        rows_a = s_own * HALF_N + s_loc
        rows_b = s_own * HALF_N + (s_loc - HALF_N)
        percore.append(
            dict(a=(rows_a[in_a], d_c[in_a]), b=(rows_b[~in_a], d_c[~in_a]))
        )
        for key in ("a", "b"):
            d_k = percore[-1][key][1]
            mx = int(np.bincount(d_k // P, minlength=NT).max())
            need = max(1, math.ceil(mx / P))
            if key == "a":
                MA = max(MA, need)
            else:
                MB = max(MB, need)
        n0 = c * NPC
        nreal = min(NPC, max(0, N - n0))
        if nreal > 0:
            gb = int(batch[n0])
            gmax = int(batch[n0 + nreal - 1])
            GT = max(GT, math.ceil((gmax - gb + 1) / P))

    inputs = []
    for c in range(NCORES):
        pc = percore[c]
        sgA, ohA = _edge_grids(*pc["a"], MA)
        sgB, ohB = _edge_grids(*pc["b"], MB)

        n0 = c * NPC
        nreal = min(NPC, max(0, N - n0))
        dloc = np.zeros(NPC, np.float32)
        dloc[:nreal] = dinv[n0 : n0 + nreal]
        sloc = np.zeros(NPC, np.float32)
        sloc[:nreal] = sqdg[n0 : n0 + nreal]
        gbase = int(batch[n0]) if nreal > 0 else 0
        pg = np.full(NPC, 30000.0, np.float32)
        pg[:nreal] = (batch[n0 : n0 + nreal] - gbase).astype(np.float32)
        growidx = (
            gbase
            + np.arange(GT, dtype=np.int32)[None, :] * P
            + np.arange(P, dtype=np.int32)[:, None]
        ).astype(np.int32)

        inputs.append(
            dict(
                xhata=xhatA, xhatb=xhatB,
                srcga=sgA, oha=ohA, srcgb=sgB, ohb=ohB,
                dinvrow=dloc.astype(F16).reshape(1, NPC),
                dinv2bc=np.ascontiguousarray(
                    np.broadcast_to(
                        (dloc.astype(np.float64) ** 2).astype(F16)[None, :],
                        (HID, NPC),
                    )
                ),
                sqdgrow=sloc.astype(F16).reshape(1, NPC),
                dinvcol=np.ascontiguousarray(
                    dloc.reshape(NT, P).T.astype(np.float32)
                ),
                poolg=np.ascontiguousarray(pg.reshape(NT, P).T),
                growidx=growidx,
            )
        )
    return inputs, MA, MB, GT


def _weights16(W1, b1, W2, b2, W3, b3, W4, b4, Wf1, bf1, Wf2):
    return dict(
        w1=np.asarray(W1, np.float32).astype(F16),
        w2=np.asarray(W2, np.float32).astype(F16),
        w3=np.asarray(W3, np.float32).astype(F16),
        w4=np.asarray(W4, np.float32).astype(F16),
        b1=np.asarray(b1, np.float32).astype(F16).reshape(1, HID),
        b2=np.asarray(b2, np.float32).astype(F16).reshape(1, HID),
        b3=np.asarray(b3, np.float32).astype(F16).reshape(1, HID),
        b4=np.asarray(b4, np.float32).astype(F16).reshape(1, HID),
        wf1=np.asarray(Wf1, np.float32).astype(F16),
        bf1col=np.asarray(bf1, np.float32).reshape(NFEAT, 1),
        wf2=np.asarray(Wf2, np.float32).astype(F16).reshape(NFEAT, 1),
    )


# ----------------------------------------------------------------------------
# Numpy emulation of the device program (index plumbing + fp16/fp8 effects).
# ----------------------------------------------------------------------------

def _emulate(inputs, MA, MB, GT, w, bf2val, dump=None):
    f32 = np.float32

    def agg_pass(table, sg, oh, M, width):
        out = np.zeros((NT, width, P), f32)
        for t in range(NT):
            for m in range(M):
                k = t * M + m
                msg = table[sg[:, k]]
                ohm = oh[:, k * P : (k + 1) * P]
                out[t] += msg.astype(f32).T @ ohm.astype(f32)
        return out

    hT = [None] * NCORES
    xhatA = inputs[0]["xhata"]
    xhatB = inputs[0]["xhatb"]
    for c in range(NCORES):
        inp = inputs[c]
        dinv2 = inp["dinv2bc"][0].astype(f32)   # f16 values
        dinvr = inp["dinvrow"][0].astype(f32)
        aggA = agg_pass(xhatA, inp["srcga"], inp["oha"], MA, NFEAT)
        aggB = agg_pass(xhatB, inp["srcgb"], inp["ohb"], MB, NFEAT)
        hTc = np.zeros((HID, NPC), F16)
        for t in range(NT):
            sl = slice(t * P, (t + 1) * P)
            aggT = ((aggA[t] + aggB[t]) * dinv2[None, sl]).astype(F16)
            ps = w["b1"].astype(f32).T @ dinvr[None, sl]
            ps = ps + w["w1"].astype(f32).T @ aggT.astype(f32)
            hTc[:, sl] = np.maximum(ps, 0).astype(F16)
        hT[c] = hTc
    if dump is not None:
        dump["h1"] = hT[0].copy()

    h4 = [None] * NCORES
    for l, wl, bl in ((2, "w2", "b2"), (3, "w3", "b3"), (4, "w4", "b4")):
        tabA = np.zeros((NAB, HID), F8)
        tabB = np.zeros((NAB, HID), F8)
        for c in range(NCORES):
            contrib = (hT[c].astype(f32).T @ w[wl].astype(f32)).astype(F8)
            tabA[c * HALF_N : (c + 1) * HALF_N] = contrib[:HALF_N]
            tabB[c * HALF_N : (c + 1) * HALF_N] = contrib[HALF_N:]
        newTs = [None] * NCORES
        for c in range(NCORES):
            inp = inputs[c]
            dinv2 = inp["dinv2bc"][0].astype(f32)
            sqdg = inp["sqdgrow"][0].astype(f32)
            dinvc = inp["dinvcol"]  # [P, NT] f32
            accA = agg_pass(tabA, inp["srcga"], inp["oha"], MA, HID).astype(F16)
            accB = agg_pass(tabB, inp["srcgb"], inp["ohb"], MB, HID)
            newT = np.zeros((HID, NPC), F16)
            newU = np.zeros((NPC, HID), F16)
            for t in range(NT):
                sl = slice(t * P, (t + 1) * P)
                bias = w[bl].astype(f32).T @ sqdg[None, sl]  # [96, P]
                tot = bias + accA[t].astype(f32) + accB[t]
                if l < 4:
                    tot = tot * dinv2[None, sl]
                    newT[:, sl] = np.maximum(tot, 0).astype(F16)
                else:
                    tot = tot * dinvc[:, t][None, :]
                    newU[sl] = np.maximum(tot, 0).astype(F16).T
            if l < 4:
                newTs[c] = newT
            else:
                h4[c] = newU
        if l < 4:
            hT = newTs
            if dump is not None:
                dump[f"h{l}"] = hT[0].copy()
    if dump is not None:
        dump["h4"] = h4[0].copy()

    zbuf = np.zeros((NZROW, NFEAT), f32)
    for c in range(NCORES):
        inp = inputs[c]
        for g in range(GT):
            gT = np.zeros((HID, P), f32)
            giota = (np.arange(P, dtype=np.float32) + g * P).astype(F16)
            for t in range(NT):
                ohp = (giota[None, :] == inp["poolg"][:, t : t + 1]).astype(F16)
                gT += h4[c][t * P : (t + 1) * P].astype(f32).T @ ohp.astype(f32)
            gT16 = gT.astype(F16)
            zpreT = w["wf1"].astype(f32).T @ gT16.astype(f32)
            zbuf[inp["growidx"][:, g]] += zpreT.T
    z = np.maximum(zbuf[:NG] + w["bf1col"].T, 0).astype(F16)
    out = z.astype(f32) @ w["wf2"].astype(f32) + bf2val
    return out.reshape(NG, 1)


# ----------------------------------------------------------------------------
# Bass program.
# ----------------------------------------------------------------------------

def _build_program(MA, MB, GT, bf2val, dump=False):
    from concourse import bacc, bass, mybir, tile
    from concourse.masks import make_identity

    f32 = mybir.dt.float32
    f16 = mybir.dt.float16
    f8 = mybir.dt.float8e4
    i32 = mybir.dt.int32
    AF = mybir.ActivationFunctionType
    OP = mybir.AluOpType

    nc = bacc.Bacc("TRN2", target_bir_lowering=False, debug=False)

    xhatA_p = nc.declare_dram_parameter("xhata", [NAB, NFEAT], f8, isOutput=False)
    xhatB_p = nc.declare_dram_parameter("xhatb", [NAB, NFEAT], f8, isOutput=False)
    sgA_p = nc.declare_dram_parameter("srcga", [P, NT * MA], i32, isOutput=False)
    ohA_p = nc.declare_dram_parameter("oha", [P, NT * MA * P], f8, isOutput=False)
    sgB_p = nc.declare_dram_parameter("srcgb", [P, NT * MB], i32, isOutput=False)
    ohB_p = nc.declare_dram_parameter("ohb", [P, NT * MB * P], f8, isOutput=False)
    dinvr_p = nc.declare_dram_parameter("dinvrow", [1, NPC], f16, isOutput=False)
    dinv2bc_p = nc.declare_dram_parameter("dinv2bc", [HID, NPC], f16, isOutput=False)
    sqdg_p = nc.declare_dram_parameter("sqdgrow", [1, NPC], f16, isOutput=False)
    dinvc_p = nc.declare_dram_parameter("dinvcol", [P, NT], f32, isOutput=False)
    poolg_p = nc.declare_dram_parameter("poolg", [P, NT], f32, isOutput=False)
    grow_p = nc.declare_dram_parameter("growidx", [P, GT], i32, isOutput=False)
    w_ps = {
        "w1": nc.declare_dram_parameter("w1", [NFEAT, HID], f16, isOutput=False),
        "w2": nc.declare_dram_parameter("w2", [HID, HID], f16, isOutput=False),
        "w3": nc.declare_dram_parameter("w3", [HID, HID], f16, isOutput=False),
        "w4": nc.declare_dram_parameter("w4", [HID, HID], f16, isOutput=False),
        "b1": nc.declare_dram_parameter("b1", [1, HID], f16, isOutput=False),
        "b2": nc.declare_dram_parameter("b2", [1, HID], f16, isOutput=False),
        "b3": nc.declare_dram_parameter("b3", [1, HID], f16, isOutput=False),
        "b4": nc.declare_dram_parameter("b4", [1, HID], f16, isOutput=False),
        "wf1": nc.declare_dram_parameter("wf1", [HID, NFEAT], f16, isOutput=False),
        "bf1col": nc.declare_dram_parameter("bf1col", [NFEAT, 1], f32, isOutput=False),
        "wf2": nc.declare_dram_parameter("wf2", [NFEAT, 1], f16, isOutput=False),
    }
    out_p = nc.declare_dram_parameter("out", [1, NG], f32, isOutput=True)
    if dump:
        dbg_ps = [
            nc.declare_dram_parameter(f"dbgh{l}", [HID, NT * P], f16, isOutput=True)
            for l in (1, 2, 3)
        ]
        dbg4_p = nc.declare_dram_parameter("dbgh4", [P, NT * HID], f16, isOutput=True)
        dbgbc_p = nc.declare_dram_parameter("dbgbc", [HID, NPC], f16, isOutput=True)

    groups = [list(range(NCORES))]
    MM = max(MA, MB)

    with tile.TileContext(nc) as tc:
        with (
            tc.tile_pool(name="const", bufs=1) as cp,
            tc.tile_pool(name="sb", bufs=1) as sb,
            tc.tile_pool(name="ps", bufs=2, space="PSUM") as ps,
            tc.tile_pool(name="dram", bufs=1, space="DRAM") as dp,
        ):
            # ---- persistent SBUF ----------------------------------------
            hT = cp.tile([HID, NT, P], f16)
            accT = cp.tile([HID, NT, P], f16)
            h4 = cp.tile([P, NT, HID], f16)
            acc4 = cp.tile([P, NT, HID], f16)
            bcast = cp.tile([HID, NPC], f16)   # dinv^2 broadcast
            sgA_sb = cp.tile([P, NT * MA], i32)
            sgB_sb = cp.tile([P, NT * MB], i32)
            dinvr_sb = cp.tile([1, NPC], f16)
            sqdg_sb = cp.tile([1, NPC], f16)
            dinvc_sb = cp.tile([P, NT], f32)
            poolg_sb = cp.tile([P, NT], f32)
            grow_sb = cp.tile([P, GT], i32)
            iota_i = cp.tile([P, GT * P], i32)
            iota_f = cp.tile([P, GT * P], f16)
            ident = cp.tile([P, P], f32)
            ones_row = cp.tile([1, P], f16)
            oneh_row = cp.tile([1, HID], f16)
            zero32 = cp.tile([P, NFEAT], f32)
            w_sb = {
                k: cp.tile(list(pshape), f16, name=k + "sb")
                for k, pshape in (
                    ("w1", (NFEAT, HID)), ("w2", (HID, HID)),
                    ("w3", (HID, HID)), ("w4", (HID, HID)),
                    ("b1", (1, HID)), ("b2", (1, HID)),
                    ("b3", (1, HID)), ("b4", (1, HID)),
                    ("wf1", (HID, NFEAT)), ("wf2", (NFEAT, 1)),
                )
            }
            bf1_sb = cp.tile([NFEAT, 1], f32)
            osb = cp.tile([1, NG], f32)

            # ---- DRAM scratch -------------------------------------------
            ctbA = [dp.tile([HALF_N, HID], f8, name=f"ctbA{l}") for l in range(3)]
            ctbB = [dp.tile([HALF_N, HID], f8, name=f"ctbB{l}") for l in range(3)]
            tabA = [
                dp.tile([NAB, HID], f8, name=f"tabA{l}", addr_space="Shared")
                for l in range(3)
            ]
            tabB = [
                dp.tile([NAB, HID], f8, name=f"tabB{l}", addr_space="Shared")
                for l in range(3)
            ]
            zbuf = dp.tile([NZROW, NFEAT], f32)
            zred = dp.tile([NZROW, NFEAT], f32, addr_space="Shared")

            # ---- load constants -----------------------------------------
            for dst_sb, src_p in (
                (sgA_sb, sgA_p), (sgB_sb, sgB_p),
                (dinvr_sb, dinvr_p), (sqdg_sb, sqdg_p),
                (dinvc_sb, dinvc_p), (poolg_sb, poolg_p), (grow_sb, grow_p),
                (bf1_sb, w_ps["bf1col"]),
            ):
                nc.sync.dma_start(out=dst_sb[:], in_=src_p[:])
            for k in w_sb:
                nc.sync.dma_start(out=w_sb[k][:], in_=w_ps[k][:])

            make_identity(nc, ident[:])
            nc.gpsimd.iota(
                iota_i[:], pattern=[[1, GT * P]], base=0, channel_multiplier=0
            )
            nc.vector.tensor_copy(out=iota_f[:], in_=iota_i[:])
            nc.vector.memset(ones_row[:], 1.0)
            nc.vector.memset(oneh_row[:], 1.0)
            nc.vector.memset(zero32[:], 0.0)

            nc.sync.dma_start(out=bcast[:], in_=dinv2bc_p[:])

            # zero the pool scatter buffer early (sync queue)
            for r in range(NZROW // P):
                nc.sync.dma_start(out=zbuf[r * P : (r + 1) * P, :], in_=zero32[:])

            def load_oh(oh_p, t, M, tag):
                oh = sb.tile([P, MM * P], f8, tag=tag, bufs=3)
                nc.sync.dma_start(
                    out=oh[:, 0 : M * P],
                    in_=oh_p[:, t * M * P : (t + 1) * M * P],
                )
                return oh

            def emit_table(li, t):
                wl = ("w2", "w3", "w4")[li]
                pc = ps.tile([P, HID], f32, tag="tbl")
                nc.tensor.matmul(
                    out=pc[:], lhsT=hT[:, t, :], rhs=w_sb[wl][:],
                    start=True, stop=True,
                )
                cs = sb.tile([P, HID], f8, tag="cs", bufs=3)
                nc.vector.tensor_copy(out=cs[:], in_=pc[:])
                if t < HALF_T:
                    nc.sync.dma_start(
                        out=ctbA[li][t * P : (t + 1) * P, :], in_=cs[:]
                    )
                else:
                    nc.sync.dma_start(
                        out=ctbB[li][(t - HALF_T) * P : (t - HALF_T + 1) * P, :],
                        in_=cs[:],
                    )
                if t == HALF_T - 1:
                    nc.gpsimd.collective_compute(
                        "AllGather", OP.bypass, replica_groups=groups,
                        ins=[ctbA[li][:]], outs=[tabA[li][:]],
                    )
                if t == NT - 1:
                    nc.gpsimd.collective_compute(
                        "AllGather", OP.bypass, replica_groups=groups,
                        ins=[ctbB[li][:]], outs=[tabB[li][:]],
                    )

            # ================= layer 1 (local chunked x-hat table) =======
            for t in range(NT):
                msgA = sb.tile([P, MM * NFEAT], f8, tag="msg1a", bufs=3)
                nc.gpsimd.indirect_dma_start(
                    out=msgA[:, 0 : MA * NFEAT],
                    out_offset=None,
                    in_=xhatA_p[:],
                    in_offset=bass.IndirectOffsetOnAxis(
                        ap=sgA_sb[:, t * MA : (t + 1) * MA], axis=0
                    ),
                )
                msgB = sb.tile([P, MM * NFEAT], f8, tag="msg1b", bufs=3)
                nc.gpsimd.indirect_dma_start(
                    out=msgB[:, 0 : MB * NFEAT],
                    out_offset=None,
                    in_=xhatB_p[:],
                    in_offset=bass.IndirectOffsetOnAxis(
                        ap=sgB_sb[:, t * MB : (t + 1) * MB], axis=0
                    ),
                )
                ohA = load_oh(ohA_p, t, MA, "oha")
                ohB = load_oh(ohB_p, t, MB, "ohb")
                agg = ps.tile([NFEAT, P], f32, tag="agg")
                nmm = MA + MB
                i = 0
                for msg, oh, M in ((msgA, ohA, MA), (msgB, ohB, MB)):
                    for m in range(M):
                        nc.tensor.matmul(
                            out=agg[:],
                            lhsT=msg[:, m * NFEAT : (m + 1) * NFEAT],
                            rhs=oh[:, m * P : (m + 1) * P],
                            start=(i == 0),
                            stop=(i == nmm - 1),
                        )
                        i += 1
                aggT = sb.tile([NFEAT, P], f16, tag="aggT", bufs=3)
                nc.vector.tensor_tensor(
                    out=aggT[:], in0=agg[:], in1=bcast[0:NFEAT, t * P : (t + 1) * P], op=OP.mult
                )
                ph = ps.tile([HID, P], f32, tag="hT")
                nc.tensor.matmul(
                    out=ph[:],
                    lhsT=w_sb["b1"][:],
                    rhs=dinvr_sb[0:1, t * P : (t + 1) * P],
                    start=True,
                    stop=False,
                )
                nc.tensor.matmul(
                    out=ph[:], lhsT=w_sb["w1"][:], rhs=aggT[:],
                    start=False, stop=True,
                )
                nc.scalar.activation(out=hT[:, t, :], in_=ph[:], func=AF.Relu)
                emit_table(0, t)
            if dump:
                for t in range(NT):
                    nc.sync.dma_start(
                        out=dbg_ps[0][:, t * P : (t + 1) * P], in_=hT[:, t, :]
                    )
                nc.sync.dma_start(out=dbgbc_p[:], in_=bcast[:])

            # ================= layers 2..4 ===============================
            for li, blname in enumerate(("b2", "b3", "b4")):
                last = li == 2
                # pass A: park raw chunk-A partial sums
                for t in range(NT):
                    msg = sb.tile([P, MM * HID], f8, tag="msg", bufs=3)
                    nc.gpsimd.indirect_dma_start(
                        out=msg[:, 0 : MA * HID],
                        out_offset=None,
                        in_=tabA[li][:],
                        in_offset=bass.IndirectOffsetOnAxis(
                            ap=sgA_sb[:, t * MA : (t + 1) * MA], axis=0
                        ),
                    )
                    oh = load_oh(ohA_p, t, MA, "oha")
                    if not last:
                        pa = ps.tile([HID, P], f32, tag="hT")
                        for m in range(MA):
                            nc.tensor.matmul(
                                out=pa[:],
                                lhsT=msg[:, m * HID : (m + 1) * HID],
                                rhs=oh[:, m * P : (m + 1) * P],
                                start=(m == 0),
                                stop=(m == MA - 1),
                            )
                        nc.vector.tensor_copy(out=accT[:, t, :], in_=pa[:])
                    else:
                        pa = ps.tile([P, HID], f32, tag="tbl")
                        for m in range(MA):
                            nc.tensor.matmul(
                                out=pa[:],
                                lhsT=oh[:, m * P : (m + 1) * P],
                                rhs=msg[:, m * HID : (m + 1) * HID],
                                start=(m == 0),
                                stop=(m == MA - 1),
                            )
                        nc.vector.tensor_copy(out=acc4[:, t, :], in_=pa[:])
                # pass B: finish; bias via PSUM-init matmul; post-scale; relu
                for t in range(NT):
                    msg = sb.tile([P, MM * HID], f8, tag="msg", bufs=3)
                    nc.gpsimd.indirect_dma_start(
                        out=msg[:, 0 : MB * HID],
                        out_offset=None,
                        in_=tabB[li][:],
                        in_offset=bass.IndirectOffsetOnAxis(
                            ap=sgB_sb[:, t * MB : (t + 1) * MB], axis=0
                        ),
                    )
                    oh = load_oh(ohB_p, t, MB, "ohb")
                    sq = sqdg_sb[0:1, t * P : (t + 1) * P]
                    if not last:
                        pb = ps.tile([HID, P], f32, tag="hT")
                        nc.tensor.matmul(
                            out=pb[:], lhsT=w_sb[blname][:], rhs=sq,
                            start=True, stop=False,
                        )
                        for m in range(MB):
                            nc.tensor.matmul(
                                out=pb[:],
                                lhsT=msg[:, m * HID : (m + 1) * HID],
                                rhs=oh[:, m * P : (m + 1) * P],
                                start=False,
                                stop=(m == MB - 1),
                            )
                        nc.vector.tensor_tensor(
                            out=pb[:], in0=pb[:], in1=accT[:, t, :], op=OP.add
                        )
                        nc.vector.tensor_tensor(
                            out=pb[:], in0=pb[:], in1=bcast[:, t * P : (t + 1) * P], op=OP.mult
                        )
                        nc.scalar.activation(
                            out=hT[:, t, :], in_=pb[:], func=AF.Relu
                        )
                        emit_table(li + 1, t)
                    else:
                        pb = ps.tile([P, HID], f32, tag="tbl")
                        nc.tensor.matmul(
                            out=pb[:], lhsT=sq, rhs=w_sb[blname][:],
                            start=True, stop=False,
                        )
                        for m in range(MB):
                            nc.tensor.matmul(
                                out=pb[:],
                                lhsT=oh[:, m * P : (m + 1) * P],
                                rhs=msg[:, m * HID : (m + 1) * HID],
                                start=False,
                                stop=(m == MB - 1),
                            )
                        nc.vector.tensor_tensor(
                            out=pb[:], in0=pb[:], in1=acc4[:, t, :], op=OP.add
                        )
                        nc.scalar.activation(
                            out=h4[:, t, :], in_=pb[:], func=AF.Relu,
                            scale=dinvc_sb[:, t : t + 1],
                        )
                if dump and not last:
                    for t in range(NT):
                        nc.sync.dma_start(
                            out=dbg_ps[li + 1][:, t * P : (t + 1) * P],
                            in_=hT[:, t, :],
                        )
            if dump:
                for t in range(NT):
                    nc.sync.dma_start(
                        out=dbg4_p[:, t * HID : (t + 1) * HID], in_=h4[:, t, :]
                    )

            # ================= pool + head ===============================
            for g in range(GT):
                pg_ps = ps.tile([HID, P], f32, tag="hT")
                for t in range(NT):
                    ohp = sb.tile([P, P], f16, tag="ohp", bufs=4)
                    nc.vector.tensor_scalar(
                        out=ohp[:],
                        in0=iota_f[:, g * P : (g + 1) * P],
                        scalar1=poolg_sb[:, t : t + 1],
                        scalar2=None,
                        op0=OP.is_equal,
                    )
                    nc.tensor.matmul(
                        out=pg_ps[:],
                        lhsT=h4[:, t, :],
                        rhs=ohp[:],
                        start=(t == 0),
                        stop=(t == NT - 1),
                    )
                gT = sb.tile([HID, P], f16, tag="gT", bufs=2)
                nc.vector.tensor_copy(out=gT[:], in_=pg_ps[:])
                pz = ps.tile([NFEAT, P], f32, tag="agg")
                nc.tensor.matmul(
                    out=pz[:], lhsT=w_sb["wf1"][:], rhs=gT[:],
                    start=True, stop=True,
                )
                zT = sb.tile([NFEAT, P], f32, tag="zT", bufs=2)
                nc.vector.tensor_copy(out=zT[:], in_=pz[:])
                pzt = ps.tile([P, NFEAT], f32, tag="tbl")
                nc.tensor.transpose(
                    out=pzt[:], in_=zT[:], identity=ident[0:NFEAT, 0:NFEAT]
                )
                zsb = sb.tile([P, NFEAT], f32, tag="zsb", bufs=2)
                nc.vector.tensor_copy(out=zsb[:], in_=pzt[:])
                nc.gpsimd.indirect_dma_start(
                    out=zbuf[:],
                    out_offset=bass.IndirectOffsetOnAxis(
                        ap=grow_sb[:, g : g + 1], axis=0
                    ),
                    in_=zsb[:],
                    in_offset=None,
                )

            nc.gpsimd.collective_compute(
                "AllReduce", OP.add, replica_groups=groups,
                ins=[zbuf[:]], outs=[zred[:]],
            )

            for j in range(NG // P):
                zr = sb.tile([P, NFEAT], f32, tag="zsb", bufs=2)
                nc.sync.dma_start(out=zr[:], in_=zred[j * P : (j + 1) * P, :])
                pt = ps.tile([NFEAT, P], f32, tag="agg")
                nc.tensor.transpose(out=pt[:], in_=zr[:], identity=ident[:])
                zrel = sb.tile([NFEAT, P], f16, tag="zrel", bufs=2)
                nc.scalar.activation(
                    out=zrel[:], in_=pt[:], func=AF.Relu, bias=bf1_sb[:, 0:1]
                )
                po = ps.tile([1, P], f32, tag="out")
                nc.tensor.matmul(
                    out=po[:], lhsT=w_sb["wf2"][:], rhs=zrel[:],
                    start=True, stop=True,
                )
                nc.vector.tensor_scalar(
                    out=osb[:, j * P : (j + 1) * P],
                    in0=po[:],
                    scalar1=float(bf2val),
                    scalar2=None,
                    op0=OP.add,
                )
            nc.sync.dma_start(out=out_p[:], in_=osb[:])

    nc.finalize()
    return nc


# ----------------------------------------------------------------------------
# Entry point.
# ----------------------------------------------------------------------------

_RUN_KWARGS = {}
LAST_RESULTS = None


def kernel(
    x, edge_index, batch,
    W1, b1, W2, b2, W3, b3, W4, b4, Wf1, bf1, Wf2, bf2,
):
    from concourse.bass_utils import run_bass_kernel_spmd

    inputs, MA, MB, GT = _prep(
        np.asarray(x), np.asarray(edge_index), np.asarray(batch)
    )
    bf2val = float(np.asarray(bf2).reshape(-1)[0])
    w16 = _weights16(W1, b1, W2, b2, W3, b3, W4, b4, Wf1, bf1, Wf2)

    if os.environ.get("BASS_EMULATE"):
        return _emulate(inputs, MA, MB, GT, w16, bf2val).astype(np.float32)

    dump = bool(os.environ.get("BASS_DEBUG_DUMP"))
    nc = _build_program(MA, MB, GT, bf2val, dump=dump)
    in_maps = [{**inputs[c], **w16} for c in range(NCORES)]
    res = run_bass_kernel_spmd(
        nc, in_maps, core_ids=list(range(NCORES)), **_RUN_KWARGS
    )
    global LAST_RESULTS
    LAST_RESULTS = res
    if dump:
        d = {}
        _emulate(inputs, MA, MB, GT, w16, bf2val, dump=d)
        gotbc = np.asarray(res.results[0]["dbgbc"]).astype(np.float32)
        wantbc = inputs[0]["dinv2bc"].astype(np.float32)
        relbc = np.linalg.norm(gotbc - wantbc) / max(np.linalg.norm(wantbc), 1e-9)
        print(f"[dump] bcast device-vs-host rel: {relbc:.3e}")
        for l in (1, 2, 3):
            got = np.asarray(res.results[0][f"dbgh{l}"]).astype(np.float32)
            wantv = d[f"h{l}"].reshape(HID, NT * P).astype(np.float32)
            rel = np.linalg.norm(got - wantv) / max(np.linalg.norm(wantv), 1e-9)
            print(f"[dump] h{l} device-vs-emulate rel: {rel:.3e}")
        got = np.asarray(res.results[0]["dbgh4"]).astype(np.float32)
        wantv = d["h4"].reshape(P, NT * HID, order="C")
        wantv = d["h4"].reshape(NT, P, HID).transpose(1, 0, 2).reshape(P, NT * HID)
        rel = np.linalg.norm(got - wantv.astype(np.float32)) / max(
            np.linalg.norm(wantv), 1e-9
        )
        print(f"[dump] h4 device-vs-emulate rel: {rel:.3e}")
    out = np.asarray(res.results[0]["out"]).reshape(NG, 1).astype(np.float32)
    return out
